# revision 10
# baseline (speedup 1.0000x reference)
import numpy as np
import ml_dtypes

import concourse.bass as bass
import concourse.bacc as bacc
import concourse.mybir as mybir
import concourse.tile as tile
from concourse import bass_utils

F32 = mybir.dt.float32
F16 = mybir.dt.float16
BF16 = mybir.dt.bfloat16
FP8 = mybir.dt.float8e4
AF = mybir.ActivationFunctionType
ALU = mybir.AluOpType

B, N, HID = 4, 4096, 128
HALF = N // 2          # rows per core
NB = N // 128          # 32 j-blocks
NSWEEP_F16 = 8         # Jacobi sweeps with fp16 matmuls (sweep 1 has no matmul)
# sweep 9 re-runs with fp32 matmuls for the final fixed point

_CACHED = {}


def build_nc(dbg=False):
    nc = bacc.Bacc("TRN2", target_bir_lowering=False, debug=False, num_devices=8)

    atf8 = nc.dram_tensor("atf8", [N, HALF], FP8, kind="ExternalInput")
    x16r = nc.dram_tensor("x16r", [128, N], F16, kind="ExternalInput")
    xdf = nc.dram_tensor("xdf", [128, 128], F32, kind="ExternalInput")
    cwt = nc.dram_tensor("cwt", [3, 128, 128], F32, kind="ExternalInput")
    wihT = nc.dram_tensor("wihT", [128, 512], F32, kind="ExternalInput")
    wihn = nc.dram_tensor("wihn", [512, 128], F32, kind="ExternalInput")
    whh16 = nc.dram_tensor("whh16", [128, 512], F16, kind="ExternalInput")
    whh32 = nc.dram_tensor("whh32", [128, 512], F32, kind="ExternalInput")
    bias4 = nc.dram_tensor("bias4", [128, 4], F32, kind="ExternalInput")
    cbb = nc.dram_tensor("cbb", [128, 128], F32, kind="ExternalInput")
    eyef = nc.dram_tensor("eyef", [128, 128], F32, kind="ExternalInput")
    mlo = nc.dram_tensor("mlo", [128, 1], F32, kind="ExternalInput")
    mhi = nc.dram_tensor("mhi", [128, 1], F32, kind="ExternalInput")
    out = nc.dram_tensor("out", [HALF, HID], F32, kind="ExternalOutput")
    if dbg:
        lw_dbg = nc.dram_tensor("lw_dbg", [128, 128], F16, kind="ExternalOutput")
        deg_dbg = nc.dram_tensor("deg_dbg", [128, 32], F32, kind="ExternalOutput")
        zx_dbg = nc.dram_tensor("zx_dbg", [128, 512], F32, kind="ExternalOutput")

    with tile.TileContext(nc) as tc:
        with (
            tc.tile_pool(name="const", bufs=1) as cp,
            tc.tile_pool(name="big", bufs=1) as bigp,
            tc.tile_pool(name="sw", bufs=2) as swp,
            tc.tile_pool(name="outs", bufs=3) as osp,
            tc.tile_pool(name="psdeg", bufs=1, space="PSUM") as psb,
            tc.tile_pool(name="psz", bufs=2, space="PSUM") as psz,
            tc.tile_pool(name="pso", bufs=2, space="PSUM") as pso,
            tc.tile_pool(name="dram", bufs=1, space="DRAM") as dram,
        ):
            # ---------- small loads (conv/Zx path first) ----------
            dfpad = cp.tile([128, 130], F32, tag="dfpad")
            nc.vector.memset(dfpad[:], 0.0)
            nc.sync.dma_start(dfpad[:, 1:129], xdf[:])
            cwt_sb = cp.tile([128, 384], F32, tag="cwt")
            for k in range(3):
                nc.sync.dma_start(cwt_sb[:, k * 128:(k + 1) * 128], cwt[k])
            wihT_sb = cp.tile([128, 512], F32, tag="wihT")
            nc.sync.dma_start(wihT_sb[:], wihT[:])
            bias4_sb = cp.tile([128, 4], F32, tag="bias4")
            nc.sync.dma_start(bias4_sb[:], bias4[:])
            cbb_sb = cp.tile([128, 128], F32, tag="cbb")
            nc.sync.dma_start(cbb_sb[:], cbb[:])
            # S[u, g] = sum_d wih_perm[g*128+u, d]
            S_sb = cp.tile([128, 4], F32, tag="S")
            for g in range(4):
                wn = swp.tile([128, 128], F32, tag="wn")
                nc.sync.dma_start(wn[:], wihn[g * 128:(g + 1) * 128, :])
                nc.vector.reduce_sum(S_sb[:, g:g + 1], wn[:], axis=mybir.AxisListType.X)

            # ---------- A^T (fp8, with +I folded in) streamed to SBUF ----------
            at_sb = bigp.tile([128, NB * 2048], FP8, tag="at")
            whh16_sb = cp.tile([128, 512], F16, tag="whh16")
            x16_sb = cp.tile([128, N], F16, tag="x16")
            whh32_sb = cp.tile([128, 512], F32, tag="whh32")
            eyef_sb = cp.tile([128, 128], F32, tag="eyef")
            mlo_sb = cp.tile([128, 1], F32, tag="mlo")
            mhi_sb = cp.tile([128, 1], F32, tag="mhi")

            def at_chunk_dma(c):
                nc.sync.dma_start(
                    at_sb[:, c * 8192:(c + 1) * 8192].rearrange("p (jb i) -> p jb i", jb=4),
                    atf8[c * 512:(c + 1) * 512, :].rearrange("(jb p) i -> p jb i", p=128),
                )

            at_chunk_dma(0)
            at_chunk_dma(1)
            nc.sync.dma_start(whh16_sb[:], whh16[:])
            nc.sync.dma_start(x16_sb[:], x16r[:])
            for c in range(2, 8):
                at_chunk_dma(c)
            nc.sync.dma_start(whh32_sb[:], whh32[:])
            nc.sync.dma_start(eyef_sb[:], eyef[:])
            nc.sync.dma_start(mlo_sb[:], mlo[:])
            nc.sync.dma_start(mhi_sb[:], mhi[:])

            ones_sb = cp.tile([128, 128], BF16, tag="ones")
            nc.vector.memset(ones_sb[:], 1.0)

            # ---------- conv -> dynT[d, t] ----------
            dyn_ps = psz.tile([128, 512], F32, tag="zps")
            for k in range(3):
                nc.tensor.matmul(
                    dyn_ps[:, 0:128], dfpad[:, k:k + 128], cwt_sb[:, k * 128:(k + 1) * 128],
                    start=(k == 0), stop=(k == 2),
                )
            dynT_sb = cp.tile([128, 128], F32, tag="dynT")
            nc.vector.tensor_copy(dynT_sb[:], dyn_ps[:, 0:128])

            # ---------- Zx[u, (g,t)] ----------
            Zx_sb = cp.tile([128, 512], F32, tag="Zx")
            zx_ps = psz.tile([128, 512], F32, tag="zps")
            for g in range(4):
                nc.tensor.matmul(
                    zx_ps[:, g * 128:(g + 1) * 128], wihT_sb[:, g * 128:(g + 1) * 128],
                    dynT_sb[:], start=True, stop=True,
                )
            for g in range(4):
                zxg = Zx_sb[:, g * 128:(g + 1) * 128]
                nc.vector.tensor_copy(zxg, zx_ps[:, g * 128:(g + 1) * 128])
                nc.vector.tensor_scalar_add(zxg, zxg, bias4_sb[:, g:g + 1])
                corr = swp.tile([128, 128], F32, tag="corr")
                nc.vector.tensor_scalar_mul(corr[:], cbb_sb[:], S_sb[:, g:g + 1])
                nc.vector.tensor_tensor(zxg, zxg, corr[:], op=ALU.add)

            # ---------- degree accumulators (PE ones-matmul over at chunks) ----------
            deg_ps = [
                psb.tile([128, 512], F32, tag=f"deg{i}", name=f"deg_ps{i}")
                for i in range(4)
            ]

            def deg_chunk_mms(c):
                for jb in range(c * 4, c * 4 + 4):
                    for sb_i in range(4):
                        nc.tensor.matmul(
                            deg_ps[sb_i][:], ones_sb[:],
                            at_sb[:, jb * 2048 + sb_i * 512:jb * 2048 + sb_i * 512 + 512],
                            start=(jb == 0), stop=(jb == NB - 1),
                            skip_group_check=True,
                        )

            # ---------- LSTM via Jacobi sweeps ----------
            H16 = cp.tile([128, 129], F16, tag="H16")
            H32 = cp.tile([128, 129], F32, tag="H32")
            nc.vector.memset(H16[:], 0.0)
            nc.vector.memset(H32[:], 0.0)

            for s in range(1, NSWEEP_F16 + 2):
                fp32s = s == NSWEEP_F16 + 1
                if s == 1:
                    zsrc = Zx_sb
                else:
                    zps = psz.tile([128, 512], F32, tag="zps")
                    lhs = whh32_sb if fp32s else whh16_sb
                    rhs = (H32 if fp32s else H16)[:, 0:128]
                    for g in range(4):
                        nc.tensor.matmul(
                            zps[:, g * 128:(g + 1) * 128],
                            lhs[:, g * 128:(g + 1) * 128], rhs,
                            start=True, stop=True,
                        )
                    deg_chunk_mms(s - 2)
                    z = swp.tile([128, 512], F32, tag="z")
                    nc.vector.tensor_tensor(z[:, 0:384], zps[:, 0:384], Zx_sb[:, 0:384], op=ALU.add)
                    nc.vector.tensor_tensor(z[:, 384:512], zps[:, 384:512], Zx_sb[:, 384:512], op=ALU.add)
                    zsrc = z
                G = swp.tile([128, 512], F32, tag="G")
                nc.scalar.activation(G[:, 0:384], zsrc[:, 0:384], AF.Sigmoid)
                nc.scalar.activation(G[:, 384:512], zsrc[:, 384:512], AF.Tanh)
                u_t = swp.tile([128, 128], F32, tag="u")
                nc.vector.tensor_tensor(u_t[:], G[:, 0:128], G[:, 384:512], op=ALU.mult)
                Ct = swp.tile([128, 128], F32, tag="C")
                nc.vector.tensor_tensor_scan(
                    Ct[:], G[:, 128:256], u_t[:], 0.0, op0=ALU.mult, op1=ALU.add
                )
                Tt = swp.tile([128, 128], F32, tag="T")
                nc.scalar.activation(Tt[:], Ct[:], AF.Tanh)
                hdst = H16 if s < NSWEEP_F16 else H32
                nc.vector.tensor_tensor(hdst[:, 1:129], G[:, 256:384], Tt[:], op=ALU.mult)

            # lw[t, u] = H32[:, 1:129]^T
            lw_ps = pso.tile([128, 128], F32, tag="outps")
            nc.tensor.transpose(lw_ps[:], H32[:, 1:129], eyef_sb[:])
            lw16 = cp.tile([128, 128], F16, tag="lw16")
            nc.vector.tensor_copy(lw16[:], lw_ps[:])

            # ---------- degree -> dinv via pair AllGather ----------
            deg_row = cp.tile([1, 2048], F32, tag="degrow")
            for sb_i in range(4):
                nc.scalar.copy(deg_row[0:1, sb_i * 512:(sb_i + 1) * 512], deg_ps[sb_i][0:1, :])
            cc_in = dram.tile([1, 2048], F32)
            cc_out = dram.tile([2, 1, 2048], F32)
            nc.gpsimd.dma_start(cc_in[:], deg_row[:])
            nc.gpsimd.collective_compute(
                "AllGather", ALU.bypass,
                replica_groups=[[0, 1], [2, 3], [4, 5], [6, 7]],
                ins=[cc_in.opt()], outs=[cc_out.opt()],
            )
            deg_all = cp.tile([128, 32], F32, tag="degall")
            nc.sync.dma_start(
                deg_all[:, 0:16].rearrange("p (o rb) -> p o rb", o=1),
                cc_out[0].rearrange("o (rb p) -> p o rb", p=128),
            )
            nc.sync.dma_start(
                deg_all[:, 16:32].rearrange("p (o rb) -> p o rb", o=1),
                cc_out[1].rearrange("o (rb p) -> p o rb", p=128),
            )

            sq = cp.tile([128, 32], F32, tag="sq")
            nc.scalar.activation(sq[:], deg_all[:], AF.Sqrt)
            dinv_all = cp.tile([128, 32], F32, tag="dinva")
            nc.vector.reciprocal(dinv_all[:], sq[:])
            t1 = cp.tile([128, 16], F32, tag="t1")
            t2 = cp.tile([128, 16], F32, tag="t2")
            nc.vector.tensor_scalar_mul(t1[:], dinv_all[:, 0:16], mlo_sb[:])
            nc.vector.tensor_scalar_mul(t2[:], dinv_all[:, 16:32], mhi_sb[:])
            dinv_own = cp.tile([128, 16], F32, tag="dinvo")
            nc.vector.tensor_tensor(dinv_own[:], t1[:], t2[:], op=ALU.add)

            if dbg:
                nc.sync.dma_start(lw_dbg[:], lw16[:])
                nc.sync.dma_start(deg_dbg[:], deg_all[:])
                nc.sync.dma_start(zx_dbg[:], Zx_sb[:])

            # ---------- Xs = dinv_j * X (fp16) ----------
            xs = cp.tile([128, N], F16, tag="xs")
            for jb in range(NB):
                nc.vector.tensor_scalar_mul(
                    xs[:, jb * 128:(jb + 1) * 128],
                    x16_sb[:, jb * 128:(jb + 1) * 128],
                    dinv_all[:, jb:jb + 1],
                )

            # ---------- aggT = Xs^T @ A_hat^T, out = sigmoid(dinv_i * aggT^T @ lw) ----------
            for sb_i in range(4):
                agg_ps = psz.tile([128, 512], F32, tag="zps")
                for jb in range(NB):
                    nc.tensor.matmul(
                        agg_ps[:], xs[:, jb * 128:(jb + 1) * 128],
                        at_sb[:, jb * 2048 + sb_i * 512:jb * 2048 + sb_i * 512 + 512],
                        start=(jb == 0), stop=(jb == NB - 1),
                        skip_group_check=True,
                    )
                aggT = osp.tile([128, 512], F16, tag="aggT")
                nc.vector.tensor_copy(aggT[:], agg_ps[:])
                for q in range(4):
                    ib = sb_i * 4 + q
                    out_ps = pso.tile([128, 128], F32, tag="outps")
                    nc.tensor.matmul(
                        out_ps[:], aggT[:, q * 128:(q + 1) * 128], lw16[:],
                        start=True, stop=True,
                    )
                    o_sb = osp.tile([128, 128], F32, tag="osb")
                    nc.scalar.activation(
                        o_sb[:], out_ps[:], AF.Sigmoid, scale=dinv_own[:, ib:ib + 1]
                    )
                    nc.sync.dma_start(out[ib * 128:(ib + 1) * 128, :], o_sb[:])
    nc.compile()
    return nc


PERM = np.concatenate([np.arange(0, 128), np.arange(128, 256),
                       np.arange(384, 512), np.arange(256, 384)])
# fp8 e4m3 encodings of 0.0, 1.0, 2.0
FP8_LUT = np.array([0x00, 0x38, 0x40], np.uint8)


def kernel(node_embedding, adjacency_matrix, conv_w, conv_b, w_ih, w_hh, b_ih, b_hh):
    if "nc" not in _CACHED:
        _CACHED["nc"] = build_nc()
    nc = _CACHED["nc"]

    X = np.asarray(node_embedding, dtype=np.float32)
    A = np.asarray(adjacency_matrix, dtype=np.float32)
    wih_p = np.asarray(w_ih, dtype=np.float32)[PERM]
    whh_p = np.asarray(w_hh, dtype=np.float32)[PERM]
    bias_p = (np.asarray(b_ih, dtype=np.float32) + np.asarray(b_hh, dtype=np.float32))[PERM]

    common = {
        "cwt": np.ascontiguousarray(np.asarray(conv_w, dtype=np.float32).transpose(2, 1, 0)),
        "wihT": np.ascontiguousarray(wih_p.T),
        "wihn": np.ascontiguousarray(wih_p),
        "whh16": np.ascontiguousarray(whh_p.T).astype(np.float16),
        "whh32": np.ascontiguousarray(whh_p.T),
        "bias4": np.ascontiguousarray(bias_p.reshape(4, 128).T),
        "cbb": np.ascontiguousarray(np.broadcast_to(np.asarray(conv_b, np.float32)[None, :], (128, 128))),
        "eyef": np.eye(128, dtype=np.float32),
    }
    ones = np.ones((128, 1), np.float32)
    zeros = np.zeros((128, 1), np.float32)

    in_maps = []
    idx = np.arange(HALF)
    for b in range(B):
        Au8 = A[b].astype(np.uint8)
        x16r = np.ascontiguousarray(
            X[b].reshape(NB, 128, 128).transpose(1, 0, 2).reshape(128, N)
        ).astype(np.float16)
        xdf = np.ascontiguousarray(X[b, N - HID:, :])
        for h in range(2):
            ATu8 = np.ascontiguousarray(Au8.T[:, h * HALF:(h + 1) * HALF])
            ATu8[h * HALF + idx, idx] += 1
            m = dict(common)
            m["atf8"] = FP8_LUT[ATu8].view(ml_dtypes.float8_e4m3)
            m["x16r"] = x16r
            m["xdf"] = xdf
            m["mlo"] = ones if h == 0 else zeros
            m["mhi"] = zeros if h == 0 else ones
            in_maps.append(m)

    _CACHED["in_maps"] = in_maps
    res = bass_utils.run_bass_kernel_spmd(nc, in_maps, core_ids=list(range(8)))

    out = np.empty((B, N, HID), np.float32)
    for c in range(8):
        b, h = c // 2, c % 2
        out[b, h * HALF:(h + 1) * HALF, :] = res.results[c]["out"]
    return out


# revision 14
# speedup vs baseline: 1.2921x; 1.2921x over previous
import numpy as np
import ml_dtypes

import concourse.bass as bass
import concourse.bacc as bacc
import concourse.mybir as mybir
import concourse.tile as tile
from concourse import bass_utils

F32 = mybir.dt.float32
F16 = mybir.dt.float16
BF16 = mybir.dt.bfloat16
FP8 = mybir.dt.float8e4
AF = mybir.ActivationFunctionType
ALU = mybir.AluOpType

B, N, HID = 4, 4096, 128
HALF = N // 2          # rows per core
NB = N // 128          # 32 j-blocks (own-half-first order)
NSWEEP_F16 = 8         # Jacobi sweeps with fp16 matmuls (sweep 1 has no matmul)
# final sweep (NSWEEP_F16+1) re-runs with fp32 matmuls

_CACHED = {}


def build_nc(dbg=False):
    nc = bacc.Bacc("TRN2", target_bir_lowering=False, debug=False, num_devices=8)

    atf8 = nc.dram_tensor("atf8", [N, HALF], FP8, kind="ExternalInput")
    x16r = nc.dram_tensor("x16r", [128, N], F16, kind="ExternalInput")
    xdf = nc.dram_tensor("xdf", [128, 128], F32, kind="ExternalInput")
    cwt = nc.dram_tensor("cwt", [3, 128, 128], F32, kind="ExternalInput")
    wihT = nc.dram_tensor("wihT", [128, 512], F32, kind="ExternalInput")
    wihn = nc.dram_tensor("wihn", [512, 128], F32, kind="ExternalInput")
    whh16 = nc.dram_tensor("whh16", [128, 512], F16, kind="ExternalInput")
    whh32 = nc.dram_tensor("whh32", [128, 512], F32, kind="ExternalInput")
    bias4 = nc.dram_tensor("bias4", [128, 4], F32, kind="ExternalInput")
    cbb = nc.dram_tensor("cbb", [128, 128], F32, kind="ExternalInput")
    eyef = nc.dram_tensor("eyef", [128, 128], F32, kind="ExternalInput")
    mlo = nc.dram_tensor("mlo", [128, 1], F32, kind="ExternalInput")
    mhi = nc.dram_tensor("mhi", [128, 1], F32, kind="ExternalInput")
    out = nc.dram_tensor("out", [HALF, HID], F32, kind="ExternalOutput")
    if dbg:
        lw_dbg = nc.dram_tensor("lw_dbg", [128, 128], F16, kind="ExternalOutput")
        deg_dbg = nc.dram_tensor("deg_dbg", [128, 32], F32, kind="ExternalOutput")
        zx_dbg = nc.dram_tensor("zx_dbg", [128, 512], F32, kind="ExternalOutput")

    with tile.TileContext(nc) as tc:
        with (
            tc.tile_pool(name="const", bufs=1) as cp,
            tc.tile_pool(name="big", bufs=1) as bigp,
            tc.tile_pool(name="sw", bufs=2) as swp,
            tc.tile_pool(name="outs", bufs=3) as osp,
            tc.tile_pool(name="psdeg", bufs=1, space="PSUM") as psb,
            tc.tile_pool(name="psz", bufs=2, space="PSUM") as psz,
            tc.tile_pool(name="pso", bufs=2, space="PSUM") as pso,
            tc.tile_pool(name="dram", bufs=1, space="DRAM") as dram,
        ):
            # ---------- warm up the collective stream with a dummy AllGather ----------
            warm_sb = cp.tile([1, 16], F32, tag="warmsb")
            nc.vector.memset(warm_sb[:], 1.0)
            cc_warm_in = dram.tile([1, 16], F32)
            cc_warm_out = dram.tile([2, 1, 16], F32)
            nc.gpsimd.dma_start(cc_warm_in[:], warm_sb[:])
            nc.gpsimd.collective_compute(
                "AllGather", ALU.bypass,
                replica_groups=[[0, 1], [2, 3], [4, 5], [6, 7]],
                ins=[cc_warm_in.opt()], outs=[cc_warm_out.opt()],
            )

            # ---------- small loads (conv/Zx path first) ----------
            dfpad = cp.tile([128, 130], F32, tag="dfpad")
            nc.vector.memset(dfpad[:], 0.0)
            nc.sync.dma_start(dfpad[:, 1:129], xdf[:])
            cwt_sb = cp.tile([128, 384], F32, tag="cwt")
            for k in range(3):
                nc.sync.dma_start(cwt_sb[:, k * 128:(k + 1) * 128], cwt[k])
            wihT_sb = cp.tile([128, 512], F32, tag="wihT")
            nc.sync.dma_start(wihT_sb[:], wihT[:])
            bias4_sb = cp.tile([128, 4], F32, tag="bias4")
            nc.sync.dma_start(bias4_sb[:], bias4[:])
            cbb_sb = cp.tile([128, 128], F32, tag="cbb")
            nc.sync.dma_start(cbb_sb[:], cbb[:])
            S_sb = cp.tile([128, 4], F32, tag="S")
            for g in range(4):
                wn = swp.tile([128, 128], F32, tag="wn")
                nc.sync.dma_start(wn[:], wihn[g * 128:(g + 1) * 128, :])
                nc.vector.reduce_sum(S_sb[:, g:g + 1], wn[:], axis=mybir.AxisListType.X)
            whh16_sb = cp.tile([128, 512], F16, tag="whh16")
            nc.sync.dma_start(whh16_sb[:], whh16[:])

            # ---------- A^T (fp8, +I folded, own j-half first) ----------
            at_sb = bigp.tile([128, NB * 2048], FP8, tag="at")

            def at_chunk_dma(c):
                nc.sync.dma_start(
                    at_sb[:, c * 8192:(c + 1) * 8192].rearrange("p (jb i) -> p jb i", jb=4),
                    atf8[c * 512:(c + 1) * 512, :].rearrange("(jb p) i -> p jb i", p=128),
                )

            for c in range(8):
                at_chunk_dma(c)
            x16_sb = cp.tile([128, N], F16, tag="x16")
            nc.sync.dma_start(x16_sb[:], x16r[:])
            whh32_sb = cp.tile([128, 512], F32, tag="whh32")
            nc.sync.dma_start(whh32_sb[:], whh32[:])
            eyef_sb = cp.tile([128, 128], F32, tag="eyef")
            nc.sync.dma_start(eyef_sb[:], eyef[:])
            mlo_sb = cp.tile([128, 1], F32, tag="mlo")
            nc.sync.dma_start(mlo_sb[:], mlo[:])
            mhi_sb = cp.tile([128, 1], F32, tag="mhi")
            nc.sync.dma_start(mhi_sb[:], mhi[:])

            ones_sb = cp.tile([128, 128], BF16, tag="ones")
            nc.vector.memset(ones_sb[:], 1.0)

            # ---------- conv -> dynT[d, t] ----------
            dyn_ps = psz.tile([128, 512], F32, tag="zps")
            for k in range(3):
                nc.tensor.matmul(
                    dyn_ps[:, 0:128], dfpad[:, k:k + 128], cwt_sb[:, k * 128:(k + 1) * 128],
                    start=(k == 0), stop=(k == 2),
                )
            dynT_sb = cp.tile([128, 128], F32, tag="dynT")
            nc.vector.tensor_copy(dynT_sb[:], dyn_ps[:, 0:128])

            # ---------- Zx[u, (g,t)] ----------
            Zx_sb = cp.tile([128, 512], F32, tag="Zx")
            zx_ps = psz.tile([128, 512], F32, tag="zps")
            for g in range(4):
                nc.tensor.matmul(
                    zx_ps[:, g * 128:(g + 1) * 128], wihT_sb[:, g * 128:(g + 1) * 128],
                    dynT_sb[:], start=True, stop=True,
                )
            for g in range(4):
                zxg = Zx_sb[:, g * 128:(g + 1) * 128]
                nc.vector.tensor_copy(zxg, zx_ps[:, g * 128:(g + 1) * 128])
                nc.vector.tensor_scalar_add(zxg, zxg, bias4_sb[:, g:g + 1])
                corr = swp.tile([128, 128], F32, tag="corr")
                nc.vector.tensor_scalar_mul(corr[:], cbb_sb[:], S_sb[:, g:g + 1])
                nc.vector.tensor_tensor(zxg, zxg, corr[:], op=ALU.add)

            # ---------- degree accumulators ----------
            deg_ps = [
                psb.tile([128, 512], F32, tag=f"deg{i}", name=f"deg_ps{i}")
                for i in range(4)
            ]

            def deg_chunk_mms(c):
                for jb in range(c * 4, c * 4 + 4):
                    for sb_i in range(4):
                        nc.tensor.matmul(
                            deg_ps[sb_i][:], ones_sb[:],
                            at_sb[:, jb * 2048 + sb_i * 512:jb * 2048 + sb_i * 512 + 512],
                            start=(jb == 0), stop=(jb == NB - 1),
                            skip_group_check=True,
                        )

            # ---------- LSTM Jacobi sweeps, deg chunks interleaved on PE ----------
            H16 = cp.tile([128, 129], F16, tag="H16")
            H32 = cp.tile([128, 129], F32, tag="H32")
            nc.vector.memset(H16[:], 0.0)
            nc.vector.memset(H32[:], 0.0)

            deg_row = cp.tile([1, 2048], F32, tag="degrow")
            deg_dram = dram.tile([1, 2048], F32)
            cc_in = dram.tile([1, 2048], F32)
            cc_out = dram.tile([2, 1, 2048], F32)

            for s in range(1, NSWEEP_F16 + 2):
                fp32s = s == NSWEEP_F16 + 1
                if s == 1:
                    deg_chunk_mms(0)
                    deg_chunk_mms(1)
                    zsrc = Zx_sb
                else:
                    zps = psz.tile([128, 512], F32, tag="zps")
                    lhs = whh32_sb if fp32s else whh16_sb
                    rhs = (H32 if fp32s else H16)[:, 0:128]
                    for g in range(4):
                        nc.tensor.matmul(
                            zps[:, g * 128:(g + 1) * 128],
                            lhs[:, g * 128:(g + 1) * 128], rhs,
                            start=True, stop=True,
                        )
                    if s <= 7:
                        deg_chunk_mms(s)
                    z = swp.tile([128, 512], F32, tag="z")
                    nc.vector.tensor_tensor(z[:, 0:384], zps[:, 0:384], Zx_sb[:, 0:384], op=ALU.add)
                    nc.vector.tensor_tensor(z[:, 384:512], zps[:, 384:512], Zx_sb[:, 384:512], op=ALU.add)
                    zsrc = z
                G = swp.tile([128, 512], F32, tag="G")
                nc.scalar.activation(G[:, 0:384], zsrc[:, 0:384], AF.Sigmoid)
                nc.scalar.activation(G[:, 384:512], zsrc[:, 384:512], AF.Tanh)
                u_t = swp.tile([128, 128], F32, tag="u")
                nc.vector.tensor_tensor(u_t[:], G[:, 0:128], G[:, 384:512], op=ALU.mult)
                Ct = swp.tile([128, 128], F32, tag="C")
                nc.vector.tensor_tensor_scan(
                    Ct[:], G[:, 128:256], u_t[:], 0.0, op0=ALU.mult, op1=ALU.add
                )
                Tt = swp.tile([128, 128], F32, tag="T")
                nc.scalar.activation(Tt[:], Ct[:], AF.Tanh)
                hdst = H16 if s < NSWEEP_F16 else H32
                nc.vector.tensor_tensor(hdst[:, 1:129], G[:, 256:384], Tt[:], op=ALU.mult)

                if s == 7:
                    # degrees are complete (all chunks issued by sweep 7); export
                    # them now: psum -> sbuf row, then local DRAM roundtrip for own
                    # dinv and the pair AllGather for the peer half.
                    nc.scalar.copy(deg_row[0:1, 0:512], deg_ps[0][0:1, :])
                    nc.scalar.copy(deg_row[0:1, 512:1024], deg_ps[1][0:1, :])
                    nc.vector.tensor_copy(deg_row[0:1, 1024:1536], deg_ps[2][0:1, :])
                    nc.vector.tensor_copy(deg_row[0:1, 1536:2048], deg_ps[3][0:1, :])
                    nc.sync.dma_start(deg_dram[:], deg_row[:])
                    nc.gpsimd.dma_start(cc_in[:], deg_row[:])
                    nc.gpsimd.collective_compute(
                        "AllGather", ALU.bypass,
                        replica_groups=[[0, 1], [2, 3], [4, 5], [6, 7]],
                        ins=[cc_in.opt()], outs=[cc_out.opt()],
                    )

            # lw[t, u] = H32[:, 1:129]^T
            lw_ps = pso.tile([128, 128], F32, tag="outps")
            nc.tensor.transpose(lw_ps[:], H32[:, 1:129], eyef_sb[:])
            lw16 = cp.tile([128, 128], F16, tag="lw16")
            nc.vector.tensor_copy(lw16[:], lw_ps[:])

            # ---------- own-half dinv (local roundtrip; own j-blocks are 0..15) ----------
            deg_own = cp.tile([128, 16], F32, tag="degown")
            nc.sync.dma_start(
                deg_own[:].rearrange("p (o rb) -> p o rb", o=1),
                deg_dram[:].rearrange("o (rb p) -> p o rb", p=128),
            )
            dinv_all = cp.tile([128, 32], F32, tag="dinva")
            sq_own = cp.tile([128, 16], F32, tag="sqown")
            nc.scalar.activation(sq_own[:], deg_own[:], AF.Sqrt)
            nc.vector.reciprocal(dinv_all[:, 0:16], sq_own[:])

            # Xs (own half) = dinv_j * X
            xs = cp.tile([128, N], F16, tag="xs")
            for jb in range(16):
                nc.vector.tensor_scalar_mul(
                    xs[:, jb * 128:(jb + 1) * 128],
                    x16_sb[:, jb * 128:(jb + 1) * 128],
                    dinv_all[:, jb:jb + 1],
                )

            # ---------- agg lo half (own j-blocks) ----------
            agg_ps = [
                psb.tile([128, 512], F32, tag=f"deg{i}", name=f"agg_ps{i}")
                for i in range(4)
            ]
            for jb in range(16):
                for sb_i in range(4):
                    nc.tensor.matmul(
                        agg_ps[sb_i][:], xs[:, jb * 128:(jb + 1) * 128],
                        at_sb[:, jb * 2048 + sb_i * 512:jb * 2048 + sb_i * 512 + 512],
                        start=(jb == 0), stop=False,
                        skip_group_check=True,
                    )

            # ---------- peer-half dinv from the AllGather ----------
            peer_raw = cp.tile([128, 32], F32, tag="peerraw")
            nc.sync.dma_start(
                peer_raw[:, 0:16].rearrange("p (o rb) -> p o rb", o=1),
                cc_out[0].rearrange("o (rb p) -> p o rb", p=128),
            )
            nc.sync.dma_start(
                peer_raw[:, 16:32].rearrange("p (o rb) -> p o rb", o=1),
                cc_out[1].rearrange("o (rb p) -> p o rb", p=128),
            )
            p1 = cp.tile([128, 16], F32, tag="p1")
            p2 = cp.tile([128, 16], F32, tag="p2")
            nc.vector.tensor_scalar_mul(p1[:], peer_raw[:, 0:16], mhi_sb[:])
            nc.vector.tensor_scalar_mul(p2[:], peer_raw[:, 16:32], mlo_sb[:])
            peer_deg = cp.tile([128, 16], F32, tag="peerdeg")
            nc.vector.tensor_tensor(peer_deg[:], p1[:], p2[:], op=ALU.add)
            sq_peer = cp.tile([128, 16], F32, tag="sqpeer")
            nc.scalar.activation(sq_peer[:], peer_deg[:], AF.Sqrt)
            nc.vector.reciprocal(dinv_all[:, 16:32], sq_peer[:])
            for jb in range(16, NB):
                nc.vector.tensor_scalar_mul(
                    xs[:, jb * 128:(jb + 1) * 128],
                    x16_sb[:, jb * 128:(jb + 1) * 128],
                    dinv_all[:, jb:jb + 1],
                )

            if dbg:
                nc.sync.dma_start(lw_dbg[:], lw16[:])
                nc.sync.dma_start(deg_dbg[:], dinv_all[:])
                nc.sync.dma_start(zx_dbg[:], Zx_sb[:])

            # ---------- agg hi half + out ----------
            for jb in range(16, NB):
                for sb_i in range(4):
                    nc.tensor.matmul(
                        agg_ps[sb_i][:], xs[:, jb * 128:(jb + 1) * 128],
                        at_sb[:, jb * 2048 + sb_i * 512:jb * 2048 + sb_i * 512 + 512],
                        start=False, stop=(jb == NB - 1),
                        skip_group_check=True,
                    )
            for sb_i in range(4):
                aggT = osp.tile([128, 512], F16, tag="aggT")
                nc.vector.tensor_copy(aggT[:], agg_ps[sb_i][:])
                for q in range(4):
                    ib = sb_i * 4 + q
                    out_ps = pso.tile([128, 128], F32, tag="outps")
                    nc.tensor.matmul(
                        out_ps[:], aggT[:, q * 128:(q + 1) * 128], lw16[:],
                        start=True, stop=True,
                    )
                    o_sb = osp.tile([128, 128], F32, tag="osb")
                    nc.scalar.activation(
                        o_sb[:], out_ps[:], AF.Sigmoid, scale=dinv_all[:, ib:ib + 1]
                    )
                    nc.sync.dma_start(out[ib * 128:(ib + 1) * 128, :], o_sb[:])
    nc.compile()
    return nc


PERM = np.concatenate([np.arange(0, 128), np.arange(128, 256),
                       np.arange(384, 512), np.arange(256, 384)])
# fp8 e4m3 encodings of 0.0, 1.0, 2.0
FP8_LUT = np.array([0x00, 0x38, 0x40], np.uint8)


def kernel(node_embedding, adjacency_matrix, conv_w, conv_b, w_ih, w_hh, b_ih, b_hh):
    if "nc" not in _CACHED:
        _CACHED["nc"] = build_nc()
    nc = _CACHED["nc"]

    X = np.asarray(node_embedding, dtype=np.float32)
    A = np.asarray(adjacency_matrix, dtype=np.float32)
    wih_p = np.asarray(w_ih, dtype=np.float32)[PERM]
    whh_p = np.asarray(w_hh, dtype=np.float32)[PERM]
    bias_p = (np.asarray(b_ih, dtype=np.float32) + np.asarray(b_hh, dtype=np.float32))[PERM]

    common = {
        "cwt": np.ascontiguousarray(np.asarray(conv_w, dtype=np.float32).transpose(2, 1, 0)),
        "wihT": np.ascontiguousarray(wih_p.T),
        "wihn": np.ascontiguousarray(wih_p),
        "whh16": np.ascontiguousarray(whh_p.T).astype(np.float16),
        "whh32": np.ascontiguousarray(whh_p.T),
        "bias4": np.ascontiguousarray(bias_p.reshape(4, 128).T),
        "cbb": np.ascontiguousarray(np.broadcast_to(np.asarray(conv_b, np.float32)[None, :], (128, 128))),
        "eyef": np.eye(128, dtype=np.float32),
    }
    ones = np.ones((128, 1), np.float32)
    zeros = np.zeros((128, 1), np.float32)

    in_maps = []
    idx = np.arange(HALF)
    for b in range(B):
        Au8 = A[b].astype(np.uint8)
        for h in range(2):
            own = slice(h * HALF, (h + 1) * HALF)
            peer = slice((1 - h) * HALF, (2 - h) * HALF)
            rows_order = np.r_[own, peer]
            # A_hat[own rows, :]^T with j (columns of the slice = rows of A_hat^T)
            # permuted own-half-first, +I on the leading diagonal block
            ATu8 = np.ascontiguousarray(Au8[own, :].T[rows_order])
            ATu8[idx, idx] += 1
            Xp = X[b][rows_order]
            x16r = np.ascontiguousarray(
                Xp.reshape(NB, 128, 128).transpose(1, 0, 2).reshape(128, N)
            ).astype(np.float16)
            m = dict(common)
            m["atf8"] = FP8_LUT[ATu8].view(ml_dtypes.float8_e4m3)
            m["x16r"] = x16r
            m["xdf"] = np.ascontiguousarray(X[b, N - HID:, :])
            m["mlo"] = ones if h == 0 else zeros
            m["mhi"] = zeros if h == 0 else ones
            in_maps.append(m)

    _CACHED["in_maps"] = in_maps
    res = bass_utils.run_bass_kernel_spmd(nc, in_maps, core_ids=list(range(8)))

    out = np.empty((B, N, HID), np.float32)
    for c in range(8):
        b, h = c // 2, c % 2
        out[b, h * HALF:(h + 1) * HALF, :] = res.results[c]["out"]
    return out


# revision 16
# speedup vs baseline: 1.4305x; 1.1071x over previous
import numpy as np
import ml_dtypes

import concourse.bass as bass
import concourse.bacc as bacc
import concourse.mybir as mybir
import concourse.tile as tile
from concourse import bass_utils

F32 = mybir.dt.float32
F32R = mybir.dt.float32r
F16 = mybir.dt.float16
BF16 = mybir.dt.bfloat16
FP8 = mybir.dt.float8e4
AF = mybir.ActivationFunctionType
ALU = mybir.AluOpType

B, N, HID = 4, 4096, 128
HALF = N // 2          # rows per core
NB = N // 128          # 32 j-blocks (own-half-first order)
NSWEEP_F16 = 8         # Jacobi sweeps with fp16 matmuls (sweep 1 has no matmul)
# final sweep (NSWEEP_F16+1) re-runs with fp32 matmuls

_CACHED = {}
import os
USE_DR = os.environ.get("K_NO_DR", "") != "1"
USE_F32R = os.environ.get("K_NO_F32R", "") != "1"


def build_nc(dbg=False):
    nc = bacc.Bacc("TRN2", target_bir_lowering=False, debug=False, num_devices=8)

    atf8 = nc.dram_tensor("atf8", [N, HALF], FP8, kind="ExternalInput")
    x16r = nc.dram_tensor("x16r", [128, N], F16, kind="ExternalInput")
    CF = F32R if USE_F32R else F32
    xdf = nc.dram_tensor("xdf", [128, 128], CF, kind="ExternalInput")
    cwt = nc.dram_tensor("cwt", [3, 128, 128], CF, kind="ExternalInput")
    wihT = nc.dram_tensor("wihT", [128, 512], CF, kind="ExternalInput")
    wihn = nc.dram_tensor("wihn", [512, 128], F32, kind="ExternalInput")
    whh16 = nc.dram_tensor("whh16", [128, 512], F16, kind="ExternalInput")
    bias4 = nc.dram_tensor("bias4", [128, 4], F32, kind="ExternalInput")
    cbb = nc.dram_tensor("cbb", [128, 128], F32, kind="ExternalInput")
    eyef = nc.dram_tensor("eyef", [128, 128], F32, kind="ExternalInput")
    mlo = nc.dram_tensor("mlo", [128, 1], F32, kind="ExternalInput")
    mhi = nc.dram_tensor("mhi", [128, 1], F32, kind="ExternalInput")
    out = nc.dram_tensor("out", [HALF, HID], F32, kind="ExternalOutput")
    if dbg:
        lw_dbg = nc.dram_tensor("lw_dbg", [128, 128], F16, kind="ExternalOutput")
        deg_dbg = nc.dram_tensor("deg_dbg", [128, 32], F32, kind="ExternalOutput")
        zx_dbg = nc.dram_tensor("zx_dbg", [128, 512], F32, kind="ExternalOutput")

    with tile.TileContext(nc) as tc:
        with (
            tc.tile_pool(name="const", bufs=1) as cp,
            tc.tile_pool(name="big", bufs=1) as bigp,
            tc.tile_pool(name="sw", bufs=2) as swp,
            tc.tile_pool(name="outs", bufs=3) as osp,
            tc.tile_pool(name="psdeg", bufs=1, space="PSUM") as psb,
            tc.tile_pool(name="psz", bufs=2, space="PSUM") as psz,
            tc.tile_pool(name="pso", bufs=2, space="PSUM") as pso,
            tc.tile_pool(name="dram", bufs=1, space="DRAM") as dram,
        ):
            # ---------- warm up the collective stream with a dummy AllGather ----------
            warm_sb = cp.tile([1, 16], F32, tag="warmsb")
            nc.vector.memset(warm_sb[:], 1.0)
            cc_warm_in = dram.tile([1, 16], F32)
            cc_warm_out = dram.tile([2, 1, 16], F32)
            nc.gpsimd.dma_start(cc_warm_in[:], warm_sb[:])
            nc.gpsimd.collective_compute(
                "AllGather", ALU.bypass,
                replica_groups=[[0, 1], [2, 3], [4, 5], [6, 7]],
                ins=[cc_warm_in.opt()], outs=[cc_warm_out.opt()],
            )

            # ---------- A^T chunk 0 first, then small loads ----------
            at_sb = bigp.tile([128, NB * 2048], FP8, tag="at")

            def at_chunk_dma(c):
                nc.sync.dma_start(
                    at_sb[:, c * 8192:(c + 1) * 8192].rearrange("p (jb i) -> p jb i", jb=4),
                    atf8[c * 512:(c + 1) * 512, :].rearrange("(jb p) i -> p jb i", p=128),
                )

            at_chunk_dma(0)
            dfpad = cp.tile([128, 130], CF, tag="dfpad")
            nc.vector.memset(dfpad[:], 0.0)
            nc.sync.dma_start(dfpad[:, 1:129], xdf[:])
            cwt_sb = cp.tile([128, 384], CF, tag="cwt")
            for k in range(3):
                nc.sync.dma_start(cwt_sb[:, k * 128:(k + 1) * 128], cwt[k])
            wihT_sb = cp.tile([128, 512], CF, tag="wihT")
            nc.sync.dma_start(wihT_sb[:], wihT[:])
            bias4_sb = cp.tile([128, 4], F32, tag="bias4")
            nc.sync.dma_start(bias4_sb[:], bias4[:])
            cbb_sb = cp.tile([128, 128], F32, tag="cbb")
            nc.sync.dma_start(cbb_sb[:], cbb[:])
            S_sb = cp.tile([128, 4], F32, tag="S")
            for g in range(4):
                wn = swp.tile([128, 128], F32, tag="wn")
                nc.sync.dma_start(wn[:], wihn[g * 128:(g + 1) * 128, :])
                nc.vector.reduce_sum(S_sb[:, g:g + 1], wn[:], axis=mybir.AxisListType.X)
            whh16_sb = cp.tile([128, 512], F16, tag="whh16")
            nc.sync.dma_start(whh16_sb[:], whh16[:])

            for c in range(1, 8):
                at_chunk_dma(c)
            x16_sb = cp.tile([128, N], F16, tag="x16")
            nc.sync.dma_start(x16_sb[:], x16r[:])
            eyef_sb = cp.tile([128, 128], F32, tag="eyef")
            nc.sync.dma_start(eyef_sb[:], eyef[:])
            mlo_sb = cp.tile([128, 1], F32, tag="mlo")
            nc.sync.dma_start(mlo_sb[:], mlo[:])
            mhi_sb = cp.tile([128, 1], F32, tag="mhi")
            nc.sync.dma_start(mhi_sb[:], mhi[:])

            ones8 = cp.tile([128, 32], FP8, tag="ones8")
            nc.vector.memset(ones8[:], 1.0)
            ones_bf = cp.tile([128, 128], BF16, tag="onesbf")
            nc.vector.memset(ones_bf[:], 1.0)

            # ---------- conv -> dynT[d, t] ----------
            dyn_ps = psz.tile([128, 512], F32, tag="zps")
            for k in range(3):
                nc.tensor.matmul(
                    dyn_ps[:, 0:128], dfpad[:, k:k + 128], cwt_sb[:, k * 128:(k + 1) * 128],
                    start=(k == 0), stop=(k == 2),
                )
            dynT_sb = cp.tile([128, 128], CF, tag="dynT")
            nc.vector.tensor_copy(dynT_sb[:], dyn_ps[:, 0:128])

            # ---------- Zx[u, (g,t)] ----------
            Zx_sb = cp.tile([128, 512], F32, tag="Zx")
            zx_ps = psz.tile([128, 512], F32, tag="zps")
            for g in range(4):
                nc.tensor.matmul(
                    zx_ps[:, g * 128:(g + 1) * 128], wihT_sb[:, g * 128:(g + 1) * 128],
                    dynT_sb[:], start=True, stop=True,
                )
            for g in range(4):
                zxg = Zx_sb[:, g * 128:(g + 1) * 128]
                nc.vector.tensor_copy(zxg, zx_ps[:, g * 128:(g + 1) * 128])
                nc.vector.tensor_scalar_add(zxg, zxg, bias4_sb[:, g:g + 1])
                corr = swp.tile([128, 128], F32, tag="corr")
                nc.vector.tensor_scalar_mul(corr[:], cbb_sb[:], S_sb[:, g:g + 1])
                nc.vector.tensor_tensor(zxg, zxg, corr[:], op=ALU.add)

            # ---------- degree accumulators ----------
            deg_ps = [
                psb.tile([128, 512], F32, tag=f"deg{i}", name=f"deg_ps{i}")
                for i in range(4)
            ]

            ones8_ap = ones8[:].rearrange("p (ko m) -> p ko m", ko=2)[:, :, 0:1]
            at3 = at_sb[:].rearrange("p (jb x) -> p jb x", jb=NB)

            def deg_chunk_mms(c):
                if USE_DR:
                    for pr in range(c * 2, c * 2 + 2):
                        for sb_i in range(4):
                            nc.tensor.matmul(
                                deg_ps[sb_i][0:1, :], ones8_ap,
                                at3[:, 2 * pr:2 * pr + 2, sb_i * 512:(sb_i + 1) * 512],
                                start=(pr == 0), stop=(pr == 15),
                                perf_mode=mybir.MatmulPerfMode.DoubleRow,
                                skip_group_check=True,
                            )
                else:
                    for jb in range(c * 4, c * 4 + 4):
                        for sb_i in range(4):
                            nc.tensor.matmul(
                                deg_ps[sb_i][:], ones_bf[:],
                                at_sb[:, jb * 2048 + sb_i * 512:jb * 2048 + sb_i * 512 + 512],
                                start=(jb == 0), stop=(jb == NB - 1),
                                skip_group_check=True,
                            )

            # ---------- LSTM Jacobi sweeps, deg chunks interleaved on PE ----------
            H16 = cp.tile([128, 129], F16, tag="H16")
            H32 = cp.tile([128, 129], F32, tag="H32")
            nc.vector.memset(H16[:], 0.0)
            nc.vector.memset(H32[:], 0.0)

            deg_row = cp.tile([1, 2048], F32, tag="degrow")
            deg_dram = dram.tile([1, 2048], F32)
            cc_in = dram.tile([1, 2048], F32)
            cc_out = dram.tile([2, 1, 2048], F32)

            for s in range(1, NSWEEP_F16 + 1):
                if s == 1:
                    deg_chunk_mms(0)
                    deg_chunk_mms(1)
                    zsrc = Zx_sb
                else:
                    zps = psz.tile([128, 512], F32, tag="zps")
                    lhs = whh16_sb
                    rhs = H16[:, 0:128]
                    for g in range(4):
                        nc.tensor.matmul(
                            zps[:, g * 128:(g + 1) * 128],
                            lhs[:, g * 128:(g + 1) * 128], rhs,
                            start=True, stop=True,
                        )
                    if s <= 7:
                        deg_chunk_mms(s)
                    z = swp.tile([128, 512], F32, tag="z")
                    nc.vector.tensor_tensor(z[:, 0:384], zps[:, 0:384], Zx_sb[:, 0:384], op=ALU.add)
                    nc.vector.tensor_tensor(z[:, 384:512], zps[:, 384:512], Zx_sb[:, 384:512], op=ALU.add)
                    zsrc = z
                G = swp.tile([128, 512], F32, tag="G")
                nc.scalar.activation(G[:, 0:384], zsrc[:, 0:384], AF.Sigmoid)
                nc.scalar.activation(G[:, 384:512], zsrc[:, 384:512], AF.Tanh)
                u_t = swp.tile([128, 128], F32, tag="u")
                nc.vector.tensor_tensor(u_t[:], G[:, 0:128], G[:, 384:512], op=ALU.mult)
                Ct = swp.tile([128, 128], F32, tag="C")
                nc.vector.tensor_tensor_scan(
                    Ct[:], G[:, 128:256], u_t[:], 0.0, op0=ALU.mult, op1=ALU.add
                )
                Tt = swp.tile([128, 128], F32, tag="T")
                nc.scalar.activation(Tt[:], Ct[:], AF.Tanh)
                hdst = H16 if s < NSWEEP_F16 else H32
                nc.vector.tensor_tensor(hdst[:, 1:129], G[:, 256:384], Tt[:], op=ALU.mult)

                if s == 7:
                    # degrees are complete (all chunks issued by sweep 7); export
                    # them now: psum -> sbuf row, then local DRAM roundtrip for own
                    # dinv and the pair AllGather for the peer half.
                    nc.scalar.copy(deg_row[0:1, 0:512], deg_ps[0][0:1, :])
                    nc.scalar.copy(deg_row[0:1, 512:1024], deg_ps[1][0:1, :])
                    nc.vector.tensor_copy(deg_row[0:1, 1024:1536], deg_ps[2][0:1, :])
                    nc.vector.tensor_copy(deg_row[0:1, 1536:2048], deg_ps[3][0:1, :])
                    nc.sync.dma_start(deg_dram[:], deg_row[:])
                    nc.gpsimd.dma_start(cc_in[:], deg_row[:])
                    nc.gpsimd.collective_compute(
                        "AllGather", ALU.bypass,
                        replica_groups=[[0, 1], [2, 3], [4, 5], [6, 7]],
                        ins=[cc_in.opt()], outs=[cc_out.opt()],
                    )

            # lw[t, u] = H32[:, 1:129]^T
            lw_ps = pso.tile([128, 128], F32, tag="outps")
            nc.tensor.transpose(lw_ps[:], H32[:, 1:129], eyef_sb[:])
            lw16 = cp.tile([128, 128], F16, tag="lw16")
            nc.vector.tensor_copy(lw16[:], lw_ps[:])

            # ---------- own-half dinv (local roundtrip; own j-blocks are 0..15) ----------
            deg_own = cp.tile([128, 16], F32, tag="degown")
            nc.sync.dma_start(
                deg_own[:].rearrange("p (o rb) -> p o rb", o=1),
                deg_dram[:].rearrange("o (rb p) -> p o rb", p=128),
            )
            dinv_all = cp.tile([128, 32], F32, tag="dinva")
            sq_own = cp.tile([128, 16], F32, tag="sqown")
            nc.scalar.activation(sq_own[:], deg_own[:], AF.Sqrt)
            nc.vector.reciprocal(dinv_all[:, 0:16], sq_own[:])

            # Xs (own half) = dinv_j * X
            xs = cp.tile([128, N], F16, tag="xs")
            for jb in range(16):
                nc.vector.tensor_scalar_mul(
                    xs[:, jb * 128:(jb + 1) * 128],
                    x16_sb[:, jb * 128:(jb + 1) * 128],
                    dinv_all[:, jb:jb + 1],
                )

            # ---------- agg lo half (own j-blocks) ----------
            agg_ps = [
                psb.tile([128, 512], F32, tag=f"deg{i}", name=f"agg_ps{i}")
                for i in range(4)
            ]
            for jb in range(16):
                for sb_i in range(4):
                    nc.tensor.matmul(
                        agg_ps[sb_i][:], xs[:, jb * 128:(jb + 1) * 128],
                        at_sb[:, jb * 2048 + sb_i * 512:jb * 2048 + sb_i * 512 + 512],
                        start=(jb == 0), stop=False,
                        skip_group_check=True,
                    )

            # ---------- peer-half dinv from the AllGather ----------
            peer_raw = cp.tile([128, 32], F32, tag="peerraw")
            nc.sync.dma_start(
                peer_raw[:, 0:16].rearrange("p (o rb) -> p o rb", o=1),
                cc_out[0].rearrange("o (rb p) -> p o rb", p=128),
            )
            nc.sync.dma_start(
                peer_raw[:, 16:32].rearrange("p (o rb) -> p o rb", o=1),
                cc_out[1].rearrange("o (rb p) -> p o rb", p=128),
            )
            p1 = cp.tile([128, 16], F32, tag="p1")
            p2 = cp.tile([128, 16], F32, tag="p2")
            nc.vector.tensor_scalar_mul(p1[:], peer_raw[:, 0:16], mhi_sb[:])
            nc.vector.tensor_scalar_mul(p2[:], peer_raw[:, 16:32], mlo_sb[:])
            peer_deg = cp.tile([128, 16], F32, tag="peerdeg")
            nc.vector.tensor_tensor(peer_deg[:], p1[:], p2[:], op=ALU.add)
            sq_peer = cp.tile([128, 16], F32, tag="sqpeer")
            nc.scalar.activation(sq_peer[:], peer_deg[:], AF.Sqrt)
            nc.vector.reciprocal(dinv_all[:, 16:32], sq_peer[:])
            for jb in range(16, NB):
                nc.vector.tensor_scalar_mul(
                    xs[:, jb * 128:(jb + 1) * 128],
                    x16_sb[:, jb * 128:(jb + 1) * 128],
                    dinv_all[:, jb:jb + 1],
                )

            if dbg:
                nc.sync.dma_start(lw_dbg[:], lw16[:])
                nc.sync.dma_start(deg_dbg[:], dinv_all[:])
                nc.sync.dma_start(zx_dbg[:], Zx_sb[:])

            # ---------- agg hi half + out (per-superblock for tail overlap) ----------
            for sb_i in range(4):
                for jb in range(16, NB):
                    nc.tensor.matmul(
                        agg_ps[sb_i][:], xs[:, jb * 128:(jb + 1) * 128],
                        at_sb[:, jb * 2048 + sb_i * 512:jb * 2048 + sb_i * 512 + 512],
                        start=False, stop=(jb == NB - 1),
                        skip_group_check=True,
                    )
                aggT = osp.tile([128, 512], F16, tag="aggT")
                nc.vector.tensor_copy(aggT[:], agg_ps[sb_i][:])
                for q in range(4):
                    ib = sb_i * 4 + q
                    out_ps = pso.tile([128, 128], F32, tag="outps")
                    nc.tensor.matmul(
                        out_ps[:], aggT[:, q * 128:(q + 1) * 128], lw16[:],
                        start=True, stop=True,
                    )
                    o_sb = osp.tile([128, 128], F32, tag="osb")
                    nc.scalar.activation(
                        o_sb[:], out_ps[:], AF.Sigmoid, scale=dinv_all[:, ib:ib + 1]
                    )
                    nc.sync.dma_start(out[ib * 128:(ib + 1) * 128, :], o_sb[:])
    nc.compile()
    return nc


PERM = np.concatenate([np.arange(0, 128), np.arange(128, 256),
                       np.arange(384, 512), np.arange(256, 384)])
# fp8 e4m3 encodings of 0.0, 1.0, 2.0
FP8_LUT = np.array([0x00, 0x38, 0x40], np.uint8)


def kernel(node_embedding, adjacency_matrix, conv_w, conv_b, w_ih, w_hh, b_ih, b_hh):
    if "nc" not in _CACHED:
        _CACHED["nc"] = build_nc()
    nc = _CACHED["nc"]

    X = np.asarray(node_embedding, dtype=np.float32)
    A = np.asarray(adjacency_matrix, dtype=np.float32)
    wih_p = np.asarray(w_ih, dtype=np.float32)[PERM]
    whh_p = np.asarray(w_hh, dtype=np.float32)[PERM]
    bias_p = (np.asarray(b_ih, dtype=np.float32) + np.asarray(b_hh, dtype=np.float32))[PERM]

    common = {
        "cwt": np.ascontiguousarray(np.asarray(conv_w, dtype=np.float32).transpose(2, 1, 0)),
        "wihT": np.ascontiguousarray(wih_p.T),
        "wihn": np.ascontiguousarray(wih_p),
        "whh16": np.ascontiguousarray(whh_p.T).astype(np.float16),
        "bias4": np.ascontiguousarray(bias_p.reshape(4, 128).T),
        "cbb": np.ascontiguousarray(np.broadcast_to(np.asarray(conv_b, np.float32)[None, :], (128, 128))),
        "eyef": np.eye(128, dtype=np.float32),
    }
    ones = np.ones((128, 1), np.float32)
    zeros = np.zeros((128, 1), np.float32)

    in_maps = []
    idx = np.arange(HALF)
    for b in range(B):
        Au8 = A[b].astype(np.uint8)
        for h in range(2):
            own = slice(h * HALF, (h + 1) * HALF)
            peer = slice((1 - h) * HALF, (2 - h) * HALF)
            rows_order = np.r_[own, peer]
            # A_hat[own rows, :]^T with j (columns of the slice = rows of A_hat^T)
            # permuted own-half-first, +I on the leading diagonal block
            ATu8 = np.ascontiguousarray(Au8[own, :].T[rows_order])
            ATu8[idx, idx] += 1
            Xp = X[b][rows_order]
            x16r = np.ascontiguousarray(
                Xp.reshape(NB, 128, 128).transpose(1, 0, 2).reshape(128, N)
            ).astype(np.float16)
            m = dict(common)
            m["atf8"] = FP8_LUT[ATu8].view(ml_dtypes.float8_e4m3)
            m["x16r"] = x16r
            m["xdf"] = np.ascontiguousarray(X[b, N - HID:, :])
            m["mlo"] = ones if h == 0 else zeros
            m["mhi"] = zeros if h == 0 else ones
            in_maps.append(m)

    _CACHED["in_maps"] = in_maps
    res = bass_utils.run_bass_kernel_spmd(nc, in_maps, core_ids=list(range(8)))

    out = np.empty((B, N, HID), np.float32)
    for c in range(8):
        b, h = c // 2, c % 2
        out[b, h * HALF:(h + 1) * HALF, :] = res.results[c]["out"]
    return out
